# revision 23
# baseline (speedup 1.0000x reference)
"""Trainium2 Bass kernel for the ContrastiveLearningModule loss.

Math (mirrors the reference):
  P = l2norm(relu(E @ W1.T + b1) @ W2.T + b2)  rowwise over [T,V,L,N,D]
  for each node type t, anchors idx[t][v,l,:]:
    pos  = sum_{(x,y) != (v,l)} exp(z . P[t,x,y,id]/TEMP)
    negi = sum_{s' != s}        exp(z . z_{s'}   /TEMP)
    negc = sum_{o,k}            exp(z . P[o,v,l,nid]/TEMP)
    loss = log(pos+negi+negc) - log(pos);  out = sum(loss)/1440

Design (v2):
 - only gathered rows are projected; each core runs TWO 768-column slots
   (vs three in v1).  The 24 (t,v,l) anchor groups pack as: cores 0-3 get
   two big groups (t<2, S=100); cores 4-7 get one big group plus one slot
   holding THREE small groups (t>=2, S=20) side by side, isolated from
   each other by block-diagonal additive masks (exp underflows to 0).
 - engine balance: relu jt0 + squares on ACT, relu jt1 + everything
   elementwise-on-768 on DVE; (z+b2) is never materialized: ACT computes
   sq=(psz+b2)^2 directly and DVE computes ph=(psz+b2)*psb in one
   scalar_tensor_tensor op.
 - norms: per-column  n2 as SIX rank-1 matmuls into a [6,128] PSUM tile
   (cost-model: ACT/DVE time ~ free-dim size, so [6,128] ln/exp beats the
   v1 [1,768] row by ~3x); rn rows broadcast back by six 128-col matmuls.
 - sims: within-type [100,100] and cross-type [100,150] land in ONE PSUM
   tile -> one mask-add, one exp(+accum) for the whole negative sum.
 - positives: per-anchor products reduce via ten free-dim-1 matmuls
   (near-free on PE) into [100,5], then one exp(+accum).
 - weights/masks DMA on the Pool (SWDGE) queue in parallel with the
   anchor-matrix DMA on the SP (HWDGE) queue.
Each core returns a partial loss sum; the host combines.
"""

import sys

import numpy as np

sys.path.insert(0, "/opt/trn_rl_repo")

import concourse.bacc as bacc
import concourse.bass as bass
import concourse.mybir as mybir
import concourse.tile as tile
from concourse.hw_specs import get_activation_tables as _real_gat

_ONE_TABLE = "natural_log_exp_and_others"  # relu/identity/exp/ln/copy/square


def _gat_one_table(arch):
    """Restrict the act-table-load pass to a single function set that covers
    every ACT func this kernel uses, so exactly one LoadActFuncSet is
    emitted."""
    tabs = _real_gat(arch)
    if _ONE_TABLE in tabs:
        return {k: (v if k == _ONE_TABLE else set()) for k, v in tabs.items()}
    return tabs


bacc.get_activation_tables = _gat_one_table

F32 = mybir.dt.float32
BF16 = mybir.dt.bfloat16
AF = mybir.ActivationFunctionType
ALU = mybir.AluOpType
NP_BF16 = mybir.dt.np(BF16)

# Problem constants (hardcoded per harness contract).
T, V, L, N, D = 4, 2, 3, 4000, 256
TEMP = 0.5
SB = 100         # anchor-block rows/cols per slot
XY = V * L       # 6 (view, layer) positions
NK = 150         # cross-negative columns per slot
GCOLS = XY * SB  # 600 anchor+positive columns per slot
SLOT = 768       # column stride per slot (600 + 150 + 18 pad)
NSLOT = 2        # slots per core
NCOL = SLOT * NSLOT  # 1536 packed columns per core
MCOL = 256       # mask columns per slot (250 sim mask + 1 ms + pad)
NCORES = 8
COUNT = 1440.0   # total anchor count in the reference loss
NEG_BIG = -15000.0  # additive mask; exp(2*(sims+NEG_BIG)) underflows to 0
MMW = 512  # max matmul moving free dim into one PSUM bank

_CACHE = {}


def _emit_body(nc, tc, consts, dram, pools, rep):
    """Stage-major emission across the two slots so each engine's in-order
    queue alternates between the slots' dependency chains (kills
    head-of-line blocking in the sims/loss tail)."""
    w1, w2, bb, ones_col, ones_colf, ones_row, idm, eps, sel, warmc, mk = consts
    xt_d, out_d = dram
    xpool, wpool, spool, psmm, psb_p, pss = pools
    r = f"r{rep}"
    w = SLOT
    halves = [slice(h0, min(h0 + MMW, w)) for h0 in range(0, w, MMW)]
    msim = mk[:, :]  # [100, NSLOT*MCOL] f32

    lm_all = spool.tile([SB, NSLOT], F32, name=f"lm{r}", tag="lm")
    xtiles = []
    for s in range(NSLOT):
        xtile = xpool.tile([128, 2 * w], BF16, name=f"xt{s}{r}", tag="xtile")
        if s == 0:  # split so the first matmul's half lands sooner
            nc.sync.dma_start(xtile[:, 0:w], xt_d[:, 0:w])
            nc.sync.dma_start(xtile[:, w:2 * w], xt_d[:, w:2 * w])
        else:
            nc.sync.dma_start(xtile[:], xt_d[:, s * 2 * w:(s + 1) * 2 * w])
        xtiles.append(xtile)

    st = [{} for _ in range(NSLOT)]

    def l1(s):
        xs = [xtiles[s][:, 0:w], xtiles[s][:, w:2 * w]]
        hs = []
        for jt in (0, 1):
            psh = psmm.tile([128, w], F32, name=f"psh{jt}", tag=f"psj{jt}")
            for sl in halves:
                nc.tensor.matmul(psh[:, sl], w1[0][:, jt * 128:(jt + 1) * 128],
                                 xs[0][:, sl], start=True, stop=False)
            for sl in halves:
                nc.tensor.matmul(psh[:, sl], w1[1][:, jt * 128:(jt + 1) * 128],
                                 xs[1][:, sl], start=False, stop=True)
            h = wpool.tile([128, w], BF16, name=f"h{jt}", tag=f"h{jt}")
            if jt == 0:
                nc.scalar.activation(h[:], psh[:], AF.Relu, bias=bb[:, 0:1])
            else:
                nc.vector.tensor_scalar(h[:], psh[:], bb[:, 1:2], 0.0,
                                        op0=ALU.add, op1=ALU.max)
            hs.append(h)
        st[s]["hs"] = hs

    def l2(s):
        hs = st[s]["hs"]
        zs, sqs = [], []
        for jt in (0, 1):
            psz = psmm.tile([128, w], F32, name=f"psz{jt}", tag=f"psj{jt}")
            for sl in halves:
                nc.tensor.matmul(psz[:, sl], w2[0][:, jt * 128:(jt + 1) * 128],
                                 hs[0][:, sl], start=True, stop=False)
                nc.tensor.matmul(psz[:, sl], w2[1][:, jt * 128:(jt + 1) * 128],
                                 hs[1][:, sl], start=False, stop=True)
            z = wpool.tile([128, w], BF16, name=f"z{jt}", tag=f"z{jt}")
            nc.scalar.activation(z[:], psz[:], AF.Identity,
                                 bias=bb[:, 2 + jt:3 + jt])
            zs.append(z)
            sq = wpool.tile([128, w], BF16, name=f"sq{jt}", tag=f"sq{jt}")
            nc.vector.tensor_mul(sq[:], z[:], z[:])
            sqs.append(sq)
        st[s]["zs"], st[s]["sqs"] = zs, sqs

    def norm_a(s):
        # n2 per column as [128,6] via free-dim-1 matmuls (one psum group
        # spanning all 12), then ln and exp on the [128,6] tile
        sqs = st[s]["sqs"]
        psn = pss.tile([128, 448], F32, name="psn", tag="psn")
        for a in range(XY):
            nc.tensor.matmul(psn[:, a:a + 1], sqs[0][:, a * 128:(a + 1) * 128],
                             ones_col[:], start=(a == 0), stop=False)
            nc.tensor.matmul(psn[:, a:a + 1], sqs[1][:, a * 128:(a + 1) * 128],
                             ones_col[:], start=False, stop=(a == XY - 1))
        lnn = wpool.tile([128, XY], F32, name="lnn", tag="lnn")
        nc.scalar.activation(lnn[:], psn[:, 0:XY], AF.Ln, bias=eps[:])
        rnT = wpool.tile([128, XY], BF16, name="rnT", tag="rnT")
        nc.scalar.activation(rnT[:], lnn[:], AF.Exp, scale=-0.5)
        st[s]["rnT"] = rnT

    def phase(s):
        # psb[p, 128c+r] = rnT[r, c] via stationary free-broadcast matmuls
        # against the identity; ph = z * psb (via an ACT copy to SBUF so
        # the DVE muls run in fast mode)
        zs, rnT = st[s]["zs"], st[s]["rnT"]
        phs = [wpool.tile([128, w], BF16, name=f"ph{jt}", tag=f"ph{jt}")
               for jt in (0, 1)]
        psbs = wpool.tile([128, w], BF16, name="psbs", tag="psbs")
        for hb in (0, 1):
            psb = psb_p.tile([128, 384], F32, name="psb", tag="psb")
            for a in range(3):
                c = hb * 3 + a
                lb = rnT[:, c:c + 1].to_broadcast([128, 128])
                nc.tensor.matmul(psb[:, a * 128:(a + 1) * 128], lb, idm[:],
                                 start=(a == 0), stop=(a == 2))
            cs = slice(hb * 384, (hb + 1) * 384)
            nc.scalar.activation(psbs[:, cs], psb[:], AF.Copy)
            for jt in (0, 1):
                nc.vector.tensor_mul(phs[jt][:, cs], zs[jt][:, cs],
                                     psbs[:, cs])
        st[s]["phs"] = phs

    def sims(s):
        phs = st[s]["phs"]
        simm = pss.tile([SB, 256], F32, name="simm", tag="simm")
        nc.tensor.matmul(simm[:, 0:SB], phs[0][:, 0:SB], phs[0][:, 0:SB],
                         start=True, stop=False)
        nc.tensor.matmul(simm[:, SB:SB + NK], phs[0][:, 0:SB],
                         phs[0][:, GCOLS:GCOLS + NK], start=False, stop=False)
        nc.tensor.matmul(simm[:, 0:SB], phs[1][:, 0:SB], phs[1][:, 0:SB],
                         start=False, stop=False)
        nc.tensor.matmul(simm[:, SB:SB + NK], phs[1][:, 0:SB],
                         phs[1][:, GCOLS:GCOLS + NK], start=False, stop=True)
        msk = spool.tile([SB, SB + NK], F32, name="msk", tag="msk")
        nc.vector.tensor_add(msk[:], simm[:, 0:SB + NK],
                             msim[:, s * MCOL:s * MCOL + SB + NK])
        eacc = spool.tile([SB, 2], F32, name="eacc", tag="eacc")
        esink = spool.tile([SB, SB + NK], F32, name="esink", tag="esink")
        nc.scalar.activation(esink[:], msk[:], AF.Exp, scale=2.0,
                             accum_out=eacc[:, 0:1])
        st[s]["eacc"] = eacc

    def pos_pr(s):
        phs = st[s]["phs"]
        prs = []
        for jt in (0, 1):
            pr = spool.tile([128, (XY - 1) * SB], BF16, name=f"pr{jt}",
                            tag=f"pr{jt}")
            zb = phs[jt][:, 0:SB].unsqueeze(1).to_broadcast([128, XY - 1, SB])
            nc.vector.tensor_mul(
                pr[:].rearrange("p (a b) -> p a b", a=XY - 1),
                phs[jt][:, SB:XY * SB].rearrange("p (a b) -> p a b", a=XY - 1),
                zb)
            prs.append(pr)
        st[s]["prs"] = prs

    def pos_mm(s):
        prs, eacc = st[s]["prs"], st[s]["eacc"]
        pprT = pss.tile([SB, 8], F32, name="pprT", tag="pprT")
        for xy in range(XY - 1):
            for jt in (0, 1):
                nc.tensor.matmul(pprT[:, xy:xy + 1],
                                 prs[jt][:, xy * SB:(xy + 1) * SB],
                                 ones_col[:],
                                 start=(xy == 0 and jt == 0),
                                 stop=(xy == XY - 2 and jt == 1))
        epos = spool.tile([SB, XY - 1], F32, name="epos", tag="epos")
        nc.scalar.activation(epos[:], pprT[:, 0:XY - 1], AF.Exp, scale=2.0,
                             accum_out=eacc[:, 1:2])

    def loss(s):
        eacc = st[s]["eacc"]
        nc.vector.tensor_add(eacc[:, 0:1], eacc[:, 0:1], eacc[:, 1:2])
        lns = spool.tile([SB, 2], F32, name="lns", tag="lns")
        nc.scalar.activation(lns[:], eacc[:], AF.Ln)
        lossv = spool.tile([SB, 1], F32, name="lossv", tag="lossv")
        nc.vector.tensor_sub(lossv[:], lns[:, 0:1], lns[:, 1:2])
        nc.vector.tensor_mul(lm_all[:, s:s + 1], lossv[:],
                             msim[:, s * MCOL + SB + NK:s * MCOL + SB + NK + 1])

    # interleaved schedule: while slot0's norm/sims chain ping-pongs on
    # ACT/DVE, the PE queue holds slot1's projection (and vice versa)
    l1(0)
    l2(0)
    l1(1)
    norm_a(0)
    phase(0)
    l2(1)
    norm_a(1)
    sims(0)
    pos_pr(0)
    pos_mm(0)
    phase(1)
    loss(0)
    sims(1)
    pos_pr(1)
    pos_mm(1)
    loss(1)

    nc.sync.dma_start(out_d[:], lm_all[:])


def _build_nc(reps=1, loop_iters=None):
    nc = bacc.Bacc("TRN2", target_bir_lowering=False, debug=False)

    xt_d = nc.dram_tensor("xt", [128, 2 * NCOL], BF16, kind="ExternalInput")
    wp_d = nc.dram_tensor("wp", [128, 4 * D], BF16, kind="ExternalInput")
    bb_d = nc.dram_tensor("bb", [128, 4], F32, kind="ExternalInput")
    mk_d = nc.dram_tensor("mk", [SB, NSLOT * MCOL], F32, kind="ExternalInput")
    idm_d = nc.dram_tensor("idm", [128, 128], BF16, kind="ExternalInput")
    sel_d = nc.dram_tensor("sel", [XY, XY * 128], BF16, kind="ExternalInput")
    out_d = nc.dram_tensor("out", [SB, NSLOT], F32, kind="ExternalOutput")

    with tile.TileContext(nc) as tc:
        with tc.tile_pool(name="const", bufs=1) as cpool:
            wtile = cpool.tile([128, 4 * D], BF16, name="wtile", tag="wtile")
            nc.sync.dma_start(wtile[:], wp_d[:])
            w1 = [wtile[:, 0:D], wtile[:, D:2 * D]]
            w2 = [wtile[:, 2 * D:3 * D], wtile[:, 3 * D:4 * D]]
            bb = cpool.tile([128, 4], F32, name="bb", tag="bb")
            nc.gpsimd.dma_start(bb[:], bb_d[:])
            mk = cpool.tile([SB, NSLOT * MCOL], F32, name="mk", tag="mk")
            nc.gpsimd.dma_start(mk[:], mk_d[:])
            ones_col = cpool.tile([128, 1], BF16, name="ones_col", tag="ones_col")
            nc.vector.memset(ones_col[:], 1.0)
            ones_colf = cpool.tile([128, 1], F32, name="ones_colf", tag="ones_colf")
            nc.vector.memset(ones_colf[:], 1.0)
            ones_row = cpool.tile([1, 128], BF16, name="ones_row", tag="ones_row")
            nc.vector.memset(ones_row[:], 1.0)
            idm = cpool.tile([128, 128], BF16, name="idm", tag="idm")
            nc.gpsimd.dma_start(idm[:], idm_d[:])
            eps = cpool.tile([128, 1], F32, name="eps", tag="eps")
            nc.vector.memset(eps[:], 1e-24)
            # PE p-state warmup: junk matmuls on a memset const while the
            # input DMAs are in flight (PE ramps to full clock after ~3us
            # of continuous work)
            warmc = cpool.tile([128, 512], BF16, name="warmc", tag="warmc")
            nc.vector.memset(warmc[:], 0.0)
            sel = cpool.tile([XY, XY * 128], BF16, name="sel", tag="sel")
            nc.gpsimd.dma_start(sel[:], sel_d[:])

            consts = (w1, w2, bb, ones_col, ones_colf, ones_row, idm, eps, sel, warmc, mk)
            dram = (xt_d, out_d)
            with (
                tc.tile_pool(name="xin", bufs=2) as xpool,
                tc.tile_pool(name="work", bufs=2) as wpool,
                tc.tile_pool(name="sbs", bufs=2) as spool,
                tc.tile_pool(name="psmm", bufs=1, space=bass.MemorySpace.PSUM) as psmm,
                tc.tile_pool(name="psbp", bufs=1, space=bass.MemorySpace.PSUM) as psb_p,
                tc.tile_pool(name="pss", bufs=1, space=bass.MemorySpace.PSUM) as pss,
            ):
                pools = (xpool, wpool, spool, psmm, psb_p, pss)
                # PE p-state warmup while the first input DMAs fly
                # (outside the loop: first iteration only)
                wps = pss.tile([128, 448], F32, name="wps", tag="psn")
                for _ in range(5):
                    nc.tensor.matmul(wps[:], warmc[:, 0:128],
                                     warmc[:, 0:448], start=True, stop=True)
                if loop_iters is not None:
                    with tc.For_i(0, loop_iters, 1,
                                  hint_engines=(mybir.EngineType.PE,
                                                mybir.EngineType.DVE,
                                                mybir.EngineType.Activation)):
                        _emit_body(nc, tc, consts, dram, pools, 0)
                else:
                    for rep in range(reps):
                        _emit_body(nc, tc, consts, dram, pools, rep)

    nc.compile()
    return nc


def _get_nc(reps=1, loop_iters=None):
    key = ("nc", reps, loop_iters)
    if key not in _CACHE:
        _CACHE[key] = _build_nc(reps, loop_iters)
    return _CACHE[key]


def _core_slots():
    """Per-core slot specs: list of (sub-groups, S_r, K_r).  Cores 0-3 get
    two big groups; cores 4-7 get one big + three smalls packed."""
    bigs = [(t, v, l) for t in (0, 1) for v in range(V) for l in range(L)]
    smalls = [(t, v, l) for t in (2, 3) for v in range(V) for l in range(L)]
    out = []
    for c in range(4):
        out.append([([bigs[2 * c]], 100, 50), ([bigs[2 * c + 1]], 100, 50)])
    for c in range(4):
        out.append([([bigs[8 + c]], 100, 50),
                    (smalls[3 * c:3 * c + 3], 20, 10)])
    return out


def _sel_mat():
    m = np.zeros((XY, XY * 128), NP_BF16)
    for a in range(XY):
        m[a, a * 128:(a + 1) * 128] = 1.0
    return np.ascontiguousarray(m)


def make_in_maps(node_embeddings, W1, b1, W2, b2, idx_prio, idx_rest,
                 neg_idx_prio, neg_idx_rest):
    E = np.asarray(node_embeddings, dtype=np.float32)
    W1 = np.asarray(W1, dtype=np.float32)
    b1 = np.asarray(b1, dtype=np.float32)
    W2 = np.asarray(W2, dtype=np.float32)
    b2 = np.asarray(b2, dtype=np.float32)
    idxs = {0: np.asarray(idx_prio)[0], 1: np.asarray(idx_prio)[1],
            2: np.asarray(idx_rest)[0], 3: np.asarray(idx_rest)[1]}
    nidxs = {0: np.asarray(neg_idx_prio)[0], 1: np.asarray(neg_idx_prio)[1],
             2: np.asarray(neg_idx_rest)[0], 3: np.asarray(neg_idx_rest)[1]}

    w1t = W1.T
    w2t = W2.T
    wp = np.concatenate([w1t[:128], w1t[128:], w2t[:128], w2t[128:]],
                        axis=1).astype(NP_BF16)
    wp = np.ascontiguousarray(wp)
    bbm = np.stack([b1[:128], b1[128:], b2[:128], b2[128:]], axis=1)
    bbm = np.ascontiguousarray(bbm, dtype=np.float32)

    in_maps = []
    for slots in _core_slots():
        X = np.zeros((NCOL, D), np.float32)
        MK = np.zeros((SB, NSLOT * MCOL), np.float32)
        for s, (subs, Sr, Kr) in enumerate(slots):
            o = s * SLOT
            mo = s * MCOL
            MK[:, mo:mo + SB + NK] = NEG_BIG
            ng = len(subs)
            R = ng * Sr          # real anchor rows/cols
            for g, (t, v, l) in enumerate(subs):
                ids = np.asarray(idxs[t][v, l])[:Sr]
                xy_list = [(v, l)] + [(x, y) for x in range(V)
                                      for y in range(L) if (x, y) != (v, l)]
                for j, (x, y) in enumerate(xy_list):
                    X[o + j * SB + g * Sr:o + j * SB + g * Sr + Sr] = \
                        E[t, x, y, ids]
                others = [u for u in range(T) if u != t]
                nb = o + GCOLS + g * 3 * Kr
                for oi, u in enumerate(others):
                    nk = np.asarray(nidxs[t][v, l, oi])[:Kr]
                    X[nb + oi * Kr:nb + (oi + 1) * Kr] = E[u, v, l, nk]
                # masks: within-type block (diag removed) + own negatives
                r0 = g * Sr
                MK[r0:r0 + Sr, mo + r0:mo + r0 + Sr] = 0.0
                MK[r0 + np.arange(Sr), mo + r0 + np.arange(Sr)] = NEG_BIG
                MK[r0:r0 + Sr, mo + SB + g * 3 * Kr:mo + SB + (g + 1) * 3 * Kr] = 0.0
                MK[r0:r0 + Sr, mo + SB + NK] = 1.0  # ms column
            # pad columns (anchor rows >= R, neg cols >= 3*Kr*ng): dup col 0
            for j in range(XY):
                X[o + j * SB + R:o + (j + 1) * SB] = X[o + j * SB]
            X[o + GCOLS + 3 * Kr * ng:o + SLOT] = X[o]
        XT = X.T.astype(NP_BF16)
        XP = np.empty((128, 2 * NCOL), NP_BF16)
        for s in range(NSLOT):
            for j in (0, 1):
                XP[:, s * 2 * SLOT + j * SLOT:(s * 2 + j + 1) * SLOT] = \
                    XT[j * 128:(j + 1) * 128, s * SLOT:(s + 1) * SLOT]
        in_maps.append({
            "xt": np.ascontiguousarray(XP),
            "wp": wp, "bb": bbm,
            "mk": np.ascontiguousarray(MK),
            "idm": np.ascontiguousarray(np.eye(128, dtype=NP_BF16)),
            "sel": _sel_mat(),
        })
    return in_maps


def _make_runner(nc):
    """Lower nc to a cached jitted SPMD executable."""
    import jax
    from jax.experimental.shard_map import shard_map
    from jax.sharding import Mesh, PartitionSpec

    from concourse import bass2jax
    from concourse import mybir as mb

    bass2jax.install_neuronx_cc_hook()
    partition_name = (nc.partition_id_tensor.name
                      if nc.partition_id_tensor else None)
    in_names, out_names, out_avals = [], [], []
    for alloc in nc.m.functions[0].allocations:
        if not isinstance(alloc, mb.MemoryLocationSet):
            continue
        name = alloc.memorylocations[0].name
        if alloc.kind == "ExternalInput":
            if name != partition_name:
                in_names.append(name)
        elif alloc.kind == "ExternalOutput":
            out_names.append(name)
            out_avals.append(jax.core.ShapedArray(
                tuple(alloc.tensor_shape), mb.dt.np(alloc.dtype)))
    n_params = len(in_names)
    n_outs = len(out_avals)
    all_in_names = list(in_names) + list(out_names)
    if partition_name is not None:
        all_in_names.append(partition_name)

    def _body(*args):
        operands = list(args)
        if partition_name is not None:
            operands.append(bass2jax.partition_id_tensor())
        return tuple(bass2jax._bass_exec_p.bind(
            *operands,
            out_avals=tuple(out_avals),
            in_names=tuple(all_in_names),
            out_names=tuple(out_names),
            lowering_input_output_aliases=(),
            sim_require_finite=True,
            sim_require_nnan=True,
            nc=nc,
        ))

    devices = jax.devices()[:NCORES]
    mesh = Mesh(np.asarray(devices), ("core",))
    donate = tuple(range(n_params, n_params + n_outs))
    sharded = jax.jit(
        shard_map(_body, mesh=mesh,
                  in_specs=(PartitionSpec("core"),) * (n_params + n_outs),
                  out_specs=(PartitionSpec("core"),) * n_outs,
                  check_rep=False),
        donate_argnums=donate, keep_unused=True)

    def run(in_maps, device_inputs=None):
        if device_inputs is None:
            device_inputs = [
                np.concatenate([np.asarray(m[name]) for m in in_maps], axis=0)
                for name in in_names]
        zeros = [np.zeros((NCORES * a.shape[0], *a.shape[1:]), a.dtype)
                 for a in out_avals]
        out_arrs = sharded(*device_inputs, *zeros)
        return [
            {name: np.asarray(out_arrs[i]).reshape(NCORES, *out_avals[i].shape)[c]
             for i, name in enumerate(out_names)}
            for c in range(NCORES)
        ]

    run.in_names = in_names
    run.mesh = mesh
    return run


def _get_runner(reps=1, loop_iters=None):
    key = ("runner", reps, loop_iters)
    if key not in _CACHE:
        _CACHE[key] = _make_runner(_get_nc(reps, loop_iters))
    return _CACHE[key]


class _Res:
    def __init__(self, results):
        self.results = results


def run_on_hw(in_maps, reps=1, device_inputs=None, loop_iters=None):
    runner = _get_runner(reps, loop_iters)
    return _Res(runner(in_maps, device_inputs=device_inputs))


def kernel(node_embeddings, W1, b1, W2, b2, idx_prio, idx_rest,
           neg_idx_prio, neg_idx_rest, num_views=2, num_layers=3):
    in_maps = make_in_maps(node_embeddings, W1, b1, W2, b2, idx_prio, idx_rest,
                           neg_idx_prio, neg_idx_rest)
    res = run_on_hw(in_maps)
    _CACHE["last_results"] = res
    total = sum(float(res.results[c]["out"].sum()) for c in range(NCORES))
    return np.float32(total / COUNT)


# revision 24
# speedup vs baseline: 1.1166x; 1.1166x over previous
"""Trainium2 Bass kernel for the ContrastiveLearningModule loss.

Math (mirrors the reference):
  P = l2norm(relu(E @ W1.T + b1) @ W2.T + b2)  rowwise over [T,V,L,N,D]
  for each node type t, anchors idx[t][v,l,:]:
    pos  = sum_{(x,y) != (v,l)} exp(z . P[t,x,y,id]/TEMP)
    negi = sum_{s' != s}        exp(z . z_{s'}   /TEMP)
    negc = sum_{o,k}            exp(z . P[o,v,l,nid]/TEMP)
    loss = log(pos+negi+negc) - log(pos);  out = sum(loss)/1440

Design (v2):
 - only gathered rows are projected; each core runs TWO 768-column slots
   (vs three in v1).  The 24 (t,v,l) anchor groups pack as: cores 0-3 get
   two big groups (t<2, S=100); cores 4-7 get one big group plus one slot
   holding THREE small groups (t>=2, S=20) side by side, isolated from
   each other by block-diagonal additive masks (exp underflows to 0).
 - engine balance: relu jt0 + squares on ACT, relu jt1 + everything
   elementwise-on-768 on DVE; (z+b2) is never materialized: ACT computes
   sq=(psz+b2)^2 directly and DVE computes ph=(psz+b2)*psb in one
   scalar_tensor_tensor op.
 - norms: per-column  n2 as SIX rank-1 matmuls into a [6,128] PSUM tile
   (cost-model: ACT/DVE time ~ free-dim size, so [6,128] ln/exp beats the
   v1 [1,768] row by ~3x); rn rows broadcast back by six 128-col matmuls.
 - sims: within-type [100,100] and cross-type [100,150] land in ONE PSUM
   tile -> one mask-add, one exp(+accum) for the whole negative sum.
 - positives: per-anchor products reduce via ten free-dim-1 matmuls
   (near-free on PE) into [100,5], then one exp(+accum).
 - weights/masks DMA on the Pool (SWDGE) queue in parallel with the
   anchor-matrix DMA on the SP (HWDGE) queue.
Each core returns a partial loss sum; the host combines.
"""

import sys

import numpy as np

sys.path.insert(0, "/opt/trn_rl_repo")

import concourse.bacc as bacc
import concourse.bass as bass
import concourse.mybir as mybir
import concourse.tile as tile
from concourse.hw_specs import get_activation_tables as _real_gat

_ONE_TABLE = "natural_log_exp_and_others"  # relu/identity/exp/ln/copy/square


def _gat_one_table(arch):
    """Restrict the act-table-load pass to a single function set that covers
    every ACT func this kernel uses, so exactly one LoadActFuncSet is
    emitted."""
    tabs = _real_gat(arch)
    if _ONE_TABLE in tabs:
        return {k: (v if k == _ONE_TABLE else set()) for k, v in tabs.items()}
    return tabs


bacc.get_activation_tables = _gat_one_table

F32 = mybir.dt.float32
BF16 = mybir.dt.bfloat16
AF = mybir.ActivationFunctionType
ALU = mybir.AluOpType
NP_BF16 = mybir.dt.np(BF16)

# Problem constants (hardcoded per harness contract).
T, V, L, N, D = 4, 2, 3, 4000, 256
TEMP = 0.5
SB = 100         # anchor-block rows/cols per slot
XY = V * L       # 6 (view, layer) positions
NK = 150         # cross-negative columns per slot
GCOLS = XY * SB  # 600 anchor+positive columns per slot
SLOT = 768       # column stride per slot (600 + 150 + 18 pad)
NSLOT = 2        # slots per core
NCOL = SLOT * NSLOT  # 1536 packed columns per core
MCOL = 256       # mask columns per slot (250 sim mask + 1 ms + pad)
NCORES = 8
COUNT = 1440.0   # total anchor count in the reference loss
NEG_BIG = -15000.0  # additive mask; exp(2*(sims+NEG_BIG)) underflows to 0
MMW = 512  # max matmul moving free dim into one PSUM bank

_CACHE = {}


def _emit_body(nc, tc, consts, dram, pools, rep):
    """Stage-major emission across the two slots so each engine's in-order
    queue alternates between the slots' dependency chains (kills
    head-of-line blocking in the sims/loss tail)."""
    w1, w2, bb, ones_col, ones_colf, ones_row, idm, eps, sel, warmc, mk = consts
    xt_d, out_d = dram
    xpool, wpool, spool, psmm, psb_p, pss = pools
    r = f"r{rep}"
    w = SLOT
    halves = [slice(h0, min(h0 + MMW, w)) for h0 in range(0, w, MMW)]
    msim = mk[:, :]  # [100, NSLOT*MCOL] f32

    lm_all = spool.tile([SB, NSLOT], F32, name=f"lm{r}", tag="lm")
    xtiles = []
    for s in range(NSLOT):
        xtile = xpool.tile([128, 2 * w], BF16, name=f"xt{s}{r}", tag="xtile")
        if s == 0:  # split so the first matmul's half lands sooner
            nc.sync.dma_start(xtile[:, 0:w], xt_d[:, 0:w])
            nc.sync.dma_start(xtile[:, w:2 * w], xt_d[:, w:2 * w])
        else:
            nc.sync.dma_start(xtile[:], xt_d[:, s * 2 * w:(s + 1) * 2 * w])
        xtiles.append(xtile)

    st = [{} for _ in range(NSLOT)]

    def l1(s):
        xs = [xtiles[s][:, 0:w], xtiles[s][:, w:2 * w]]
        hs = []
        for jt in (0, 1):
            psh = psmm.tile([128, w], F32, name=f"psh{jt}", tag=f"psj{jt}")
            for sl in halves:
                nc.tensor.matmul(psh[:, sl], w1[0][:, jt * 128:(jt + 1) * 128],
                                 xs[0][:, sl], start=True, stop=False)
                nc.tensor.matmul(psh[:, sl], w1[1][:, jt * 128:(jt + 1) * 128],
                                 xs[1][:, sl], start=False, stop=True)
            h = wpool.tile([128, w], BF16, name=f"h{jt}", tag=f"h{jt}")
            if jt == 0:
                nc.scalar.activation(h[:], psh[:], AF.Relu, bias=bb[:, 0:1])
            else:
                nc.vector.tensor_scalar(h[:], psh[:], bb[:, 1:2], 0.0,
                                        op0=ALU.add, op1=ALU.max)
            hs.append(h)
        st[s]["hs"] = hs

    def l2(s):
        hs = st[s]["hs"]
        zs, sqs = [], []
        for jt in (0, 1):
            psz = psmm.tile([128, w], F32, name=f"psz{jt}", tag=f"psj{jt}")
            for sl in halves:
                nc.tensor.matmul(psz[:, sl], w2[0][:, jt * 128:(jt + 1) * 128],
                                 hs[0][:, sl], start=True, stop=False)
                nc.tensor.matmul(psz[:, sl], w2[1][:, jt * 128:(jt + 1) * 128],
                                 hs[1][:, sl], start=False, stop=True)
            z = wpool.tile([128, w], BF16, name=f"z{jt}", tag=f"z{jt}")
            nc.scalar.activation(z[:], psz[:], AF.Identity,
                                 bias=bb[:, 2 + jt:3 + jt])
            zs.append(z)
            sq = wpool.tile([128, w], BF16, name=f"sq{jt}", tag=f"sq{jt}")
            nc.vector.tensor_mul(sq[:], z[:], z[:])
            sqs.append(sq)
        st[s]["zs"], st[s]["sqs"] = zs, sqs

    def norm_a(s):
        # n2 per column as [128,6] via free-dim-1 matmuls (one psum group
        # spanning all 12), then ln and exp on the [128,6] tile
        sqs = st[s]["sqs"]
        psn = pss.tile([128, 448], F32, name="psn", tag="psn")
        for a in range(XY):
            nc.tensor.matmul(psn[:, a:a + 1], sqs[0][:, a * 128:(a + 1) * 128],
                             ones_col[:], start=(a == 0), stop=False)
            nc.tensor.matmul(psn[:, a:a + 1], sqs[1][:, a * 128:(a + 1) * 128],
                             ones_col[:], start=False, stop=(a == XY - 1))
        lnn = wpool.tile([128, XY], F32, name="lnn", tag="lnn")
        nc.scalar.activation(lnn[:], psn[:, 0:XY], AF.Ln, bias=eps[:])
        rnT = wpool.tile([128, XY], BF16, name="rnT", tag="rnT")
        nc.scalar.activation(rnT[:], lnn[:], AF.Exp, scale=-0.5)
        st[s]["rnT"] = rnT

    def phase(s):
        # psb[p, 128c+r] = rnT[r, c] via stationary free-broadcast matmuls
        # against the identity; ph = z * psb (via an ACT copy to SBUF so
        # the DVE muls run in fast mode)
        zs, rnT = st[s]["zs"], st[s]["rnT"]
        phs = [wpool.tile([128, w], BF16, name=f"ph{jt}", tag=f"ph{jt}")
               for jt in (0, 1)]
        psbs = wpool.tile([128, w], BF16, name="psbs", tag="psbs")
        for hb in (0, 1):
            psb = psb_p.tile([128, 384], F32, name="psb", tag="psb")
            for a in range(3):
                c = hb * 3 + a
                lb = rnT[:, c:c + 1].to_broadcast([128, 128])
                nc.tensor.matmul(psb[:, a * 128:(a + 1) * 128], lb, idm[:],
                                 start=(a == 0), stop=(a == 2))
            cs = slice(hb * 384, (hb + 1) * 384)
            nc.scalar.activation(psbs[:, cs], psb[:], AF.Copy)
            for jt in (0, 1):
                nc.vector.tensor_mul(phs[jt][:, cs], zs[jt][:, cs],
                                     psbs[:, cs])
        st[s]["phs"] = phs

    def sims(s):
        phs = st[s]["phs"]
        simm = pss.tile([SB, 256], F32, name="simm", tag="simm")
        nc.tensor.matmul(simm[:, 0:SB], phs[0][:, 0:SB], phs[0][:, 0:SB],
                         start=True, stop=False)
        nc.tensor.matmul(simm[:, SB:SB + NK], phs[0][:, 0:SB],
                         phs[0][:, GCOLS:GCOLS + NK], start=False, stop=False)
        nc.tensor.matmul(simm[:, 0:SB], phs[1][:, 0:SB], phs[1][:, 0:SB],
                         start=False, stop=False)
        nc.tensor.matmul(simm[:, SB:SB + NK], phs[1][:, 0:SB],
                         phs[1][:, GCOLS:GCOLS + NK], start=False, stop=True)
        msk = spool.tile([SB, SB + NK], F32, name="msk", tag="msk")
        nc.vector.tensor_add(msk[:], simm[:, 0:SB + NK],
                             msim[:, s * MCOL:s * MCOL + SB + NK])
        eacc = spool.tile([SB, 2], F32, name="eacc", tag="eacc")
        esink = spool.tile([SB, SB + NK], F32, name="esink", tag="esink")
        nc.scalar.activation(esink[:], msk[:], AF.Exp, scale=2.0,
                             accum_out=eacc[:, 0:1])
        st[s]["eacc"] = eacc

    def pos_pr(s):
        phs = st[s]["phs"]
        prs = []
        for jt in (0, 1):
            pr = spool.tile([128, (XY - 1) * SB], BF16, name=f"pr{jt}",
                            tag=f"pr{jt}")
            zb = phs[jt][:, 0:SB].unsqueeze(1).to_broadcast([128, XY - 1, SB])
            nc.vector.tensor_mul(
                pr[:].rearrange("p (a b) -> p a b", a=XY - 1),
                phs[jt][:, SB:XY * SB].rearrange("p (a b) -> p a b", a=XY - 1),
                zb)
            prs.append(pr)
        st[s]["prs"] = prs

    def pos_mm(s):
        prs, eacc = st[s]["prs"], st[s]["eacc"]
        pprT = pss.tile([SB, 8], F32, name="pprT", tag="pprT")
        for xy in range(XY - 1):
            for jt in (0, 1):
                nc.tensor.matmul(pprT[:, xy:xy + 1],
                                 prs[jt][:, xy * SB:(xy + 1) * SB],
                                 ones_col[:],
                                 start=(xy == 0 and jt == 0),
                                 stop=(xy == XY - 2 and jt == 1))
        epos = spool.tile([SB, XY - 1], F32, name="epos", tag="epos")
        nc.scalar.activation(epos[:], pprT[:, 0:XY - 1], AF.Exp, scale=2.0,
                             accum_out=eacc[:, 1:2])

    def loss(s):
        eacc = st[s]["eacc"]
        nc.vector.tensor_add(eacc[:, 0:1], eacc[:, 0:1], eacc[:, 1:2])
        lns = spool.tile([SB, 2], F32, name="lns", tag="lns")
        nc.scalar.activation(lns[:], eacc[:], AF.Ln)
        lossv = spool.tile([SB, 1], F32, name="lossv", tag="lossv")
        nc.vector.tensor_sub(lossv[:], lns[:, 0:1], lns[:, 1:2])
        nc.vector.tensor_mul(lm_all[:, s:s + 1], lossv[:],
                             msim[:, s * MCOL + SB + NK:s * MCOL + SB + NK + 1])

    # interleaved schedule: while slot0's norm/sims chain ping-pongs on
    # ACT/DVE, the PE queue holds slot1's projection (and vice versa)
    l1(0)
    l2(0)
    l1(1)
    norm_a(0)
    phase(0)
    l2(1)
    norm_a(1)
    sims(0)
    pos_pr(0)
    pos_mm(0)
    phase(1)
    loss(0)
    sims(1)
    pos_pr(1)
    pos_mm(1)
    loss(1)

    nc.sync.dma_start(out_d[:], lm_all[:])


def _build_nc(reps=1, loop_iters=None):
    nc = bacc.Bacc("TRN2", target_bir_lowering=False, debug=False)

    xt_d = nc.dram_tensor("xt", [128, 2 * NCOL], BF16, kind="ExternalInput")
    wp_d = nc.dram_tensor("wp", [128, 4 * D], BF16, kind="ExternalInput")
    bb_d = nc.dram_tensor("bb", [128, 4], F32, kind="ExternalInput")
    mk_d = nc.dram_tensor("mk", [SB, NSLOT * MCOL], F32, kind="ExternalInput")
    idm_d = nc.dram_tensor("idm", [128, 128], BF16, kind="ExternalInput")
    sel_d = nc.dram_tensor("sel", [XY, XY * 128], BF16, kind="ExternalInput")
    out_d = nc.dram_tensor("out", [SB, NSLOT], F32, kind="ExternalOutput")

    with tile.TileContext(nc) as tc:
        with tc.tile_pool(name="const", bufs=1) as cpool:
            wtile = cpool.tile([128, 4 * D], BF16, name="wtile", tag="wtile")
            nc.sync.dma_start(wtile[:], wp_d[:])
            w1 = [wtile[:, 0:D], wtile[:, D:2 * D]]
            w2 = [wtile[:, 2 * D:3 * D], wtile[:, 3 * D:4 * D]]
            bb = cpool.tile([128, 4], F32, name="bb", tag="bb")
            nc.gpsimd.dma_start(bb[:], bb_d[:])
            mk = cpool.tile([SB, NSLOT * MCOL], F32, name="mk", tag="mk")
            nc.gpsimd.dma_start(mk[:], mk_d[:])
            ones_col = cpool.tile([128, 1], BF16, name="ones_col", tag="ones_col")
            nc.vector.memset(ones_col[:], 1.0)
            ones_colf = cpool.tile([128, 1], F32, name="ones_colf", tag="ones_colf")
            nc.vector.memset(ones_colf[:], 1.0)
            ones_row = cpool.tile([1, 128], BF16, name="ones_row", tag="ones_row")
            nc.vector.memset(ones_row[:], 1.0)
            idm = cpool.tile([128, 128], BF16, name="idm", tag="idm")
            nc.gpsimd.dma_start(idm[:], idm_d[:])
            eps = cpool.tile([128, 1], F32, name="eps", tag="eps")
            nc.vector.memset(eps[:], 1e-24)
            # PE p-state warmup: junk matmuls on a memset const while the
            # input DMAs are in flight (PE ramps to full clock after ~3us
            # of continuous work)
            warmc = cpool.tile([128, 512], BF16, name="warmc", tag="warmc")
            nc.vector.memset(warmc[:], 0.0)
            sel = cpool.tile([XY, XY * 128], BF16, name="sel", tag="sel")
            nc.gpsimd.dma_start(sel[:], sel_d[:])

            consts = (w1, w2, bb, ones_col, ones_colf, ones_row, idm, eps, sel, warmc, mk)
            dram = (xt_d, out_d)
            with (
                tc.tile_pool(name="xin", bufs=2) as xpool,
                tc.tile_pool(name="work", bufs=2) as wpool,
                tc.tile_pool(name="sbs", bufs=2) as spool,
                tc.tile_pool(name="psmm", bufs=1, space=bass.MemorySpace.PSUM) as psmm,
                tc.tile_pool(name="psbp", bufs=1, space=bass.MemorySpace.PSUM) as psb_p,
                tc.tile_pool(name="pss", bufs=1, space=bass.MemorySpace.PSUM) as pss,
            ):
                pools = (xpool, wpool, spool, psmm, psb_p, pss)
                # PE p-state warmup while the first input DMAs fly
                # (outside the loop: first iteration only)
                wps = pss.tile([128, 448], F32, name="wps", tag="psn")
                for _ in range(5):
                    nc.tensor.matmul(wps[:], warmc[:, 0:128],
                                     warmc[:, 0:448], start=True, stop=True)
                if loop_iters is not None:
                    with tc.For_i(0, loop_iters, 1,
                                  hint_engines=(mybir.EngineType.PE,
                                                mybir.EngineType.DVE,
                                                mybir.EngineType.Activation)):
                        _emit_body(nc, tc, consts, dram, pools, 0)
                else:
                    for rep in range(reps):
                        _emit_body(nc, tc, consts, dram, pools, rep)

    nc.compile()
    return nc


def _get_nc(reps=1, loop_iters=None):
    key = ("nc", reps, loop_iters)
    if key not in _CACHE:
        _CACHE[key] = _build_nc(reps, loop_iters)
    return _CACHE[key]


def _core_slots():
    """Per-core slot specs: list of (sub-groups, S_r, K_r).  Cores 0-3 get
    two big groups; cores 4-7 get one big + three smalls packed."""
    bigs = [(t, v, l) for t in (0, 1) for v in range(V) for l in range(L)]
    smalls = [(t, v, l) for t in (2, 3) for v in range(V) for l in range(L)]
    out = []
    for c in range(4):
        out.append([([bigs[2 * c]], 100, 50), ([bigs[2 * c + 1]], 100, 50)])
    for c in range(4):
        out.append([([bigs[8 + c]], 100, 50),
                    (smalls[3 * c:3 * c + 3], 20, 10)])
    return out


def _sel_mat():
    m = np.zeros((XY, XY * 128), NP_BF16)
    for a in range(XY):
        m[a, a * 128:(a + 1) * 128] = 1.0
    return np.ascontiguousarray(m)


def make_in_maps(node_embeddings, W1, b1, W2, b2, idx_prio, idx_rest,
                 neg_idx_prio, neg_idx_rest):
    E = np.asarray(node_embeddings, dtype=np.float32)
    W1 = np.asarray(W1, dtype=np.float32)
    b1 = np.asarray(b1, dtype=np.float32)
    W2 = np.asarray(W2, dtype=np.float32)
    b2 = np.asarray(b2, dtype=np.float32)
    idxs = {0: np.asarray(idx_prio)[0], 1: np.asarray(idx_prio)[1],
            2: np.asarray(idx_rest)[0], 3: np.asarray(idx_rest)[1]}
    nidxs = {0: np.asarray(neg_idx_prio)[0], 1: np.asarray(neg_idx_prio)[1],
             2: np.asarray(neg_idx_rest)[0], 3: np.asarray(neg_idx_rest)[1]}

    w1t = W1.T
    w2t = W2.T
    wp = np.concatenate([w1t[:128], w1t[128:], w2t[:128], w2t[128:]],
                        axis=1).astype(NP_BF16)
    wp = np.ascontiguousarray(wp)
    bbm = np.stack([b1[:128], b1[128:], b2[:128], b2[128:]], axis=1)
    bbm = np.ascontiguousarray(bbm, dtype=np.float32)

    in_maps = []
    for slots in _core_slots():
        X = np.zeros((NCOL, D), np.float32)
        MK = np.zeros((SB, NSLOT * MCOL), np.float32)
        for s, (subs, Sr, Kr) in enumerate(slots):
            o = s * SLOT
            mo = s * MCOL
            MK[:, mo:mo + SB + NK] = NEG_BIG
            ng = len(subs)
            R = ng * Sr          # real anchor rows/cols
            for g, (t, v, l) in enumerate(subs):
                ids = np.asarray(idxs[t][v, l])[:Sr]
                xy_list = [(v, l)] + [(x, y) for x in range(V)
                                      for y in range(L) if (x, y) != (v, l)]
                for j, (x, y) in enumerate(xy_list):
                    X[o + j * SB + g * Sr:o + j * SB + g * Sr + Sr] = \
                        E[t, x, y, ids]
                others = [u for u in range(T) if u != t]
                nb = o + GCOLS + g * 3 * Kr
                for oi, u in enumerate(others):
                    nk = np.asarray(nidxs[t][v, l, oi])[:Kr]
                    X[nb + oi * Kr:nb + (oi + 1) * Kr] = E[u, v, l, nk]
                # masks: within-type block (diag removed) + own negatives
                r0 = g * Sr
                MK[r0:r0 + Sr, mo + r0:mo + r0 + Sr] = 0.0
                MK[r0 + np.arange(Sr), mo + r0 + np.arange(Sr)] = NEG_BIG
                MK[r0:r0 + Sr, mo + SB + g * 3 * Kr:mo + SB + (g + 1) * 3 * Kr] = 0.0
                MK[r0:r0 + Sr, mo + SB + NK] = 1.0  # ms column
            # pad columns (anchor rows >= R, neg cols >= 3*Kr*ng): dup col 0
            for j in range(XY):
                X[o + j * SB + R:o + (j + 1) * SB] = X[o + j * SB]
            X[o + GCOLS + 3 * Kr * ng:o + SLOT] = X[o]
        XT = X.T.astype(NP_BF16)
        XP = np.empty((128, 2 * NCOL), NP_BF16)
        for s in range(NSLOT):
            for j in (0, 1):
                XP[:, s * 2 * SLOT + j * SLOT:(s * 2 + j + 1) * SLOT] = \
                    XT[j * 128:(j + 1) * 128, s * SLOT:(s + 1) * SLOT]
        in_maps.append({
            "xt": np.ascontiguousarray(XP),
            "wp": wp, "bb": bbm,
            "mk": np.ascontiguousarray(MK),
            "idm": np.ascontiguousarray(np.eye(128, dtype=NP_BF16)),
            "sel": _sel_mat(),
        })
    return in_maps


def _make_runner(nc):
    """Lower nc to a cached jitted SPMD executable."""
    import jax
    from jax.experimental.shard_map import shard_map
    from jax.sharding import Mesh, PartitionSpec

    from concourse import bass2jax
    from concourse import mybir as mb

    bass2jax.install_neuronx_cc_hook()
    partition_name = (nc.partition_id_tensor.name
                      if nc.partition_id_tensor else None)
    in_names, out_names, out_avals = [], [], []
    for alloc in nc.m.functions[0].allocations:
        if not isinstance(alloc, mb.MemoryLocationSet):
            continue
        name = alloc.memorylocations[0].name
        if alloc.kind == "ExternalInput":
            if name != partition_name:
                in_names.append(name)
        elif alloc.kind == "ExternalOutput":
            out_names.append(name)
            out_avals.append(jax.core.ShapedArray(
                tuple(alloc.tensor_shape), mb.dt.np(alloc.dtype)))
    n_params = len(in_names)
    n_outs = len(out_avals)
    all_in_names = list(in_names) + list(out_names)
    if partition_name is not None:
        all_in_names.append(partition_name)

    def _body(*args):
        operands = list(args)
        if partition_name is not None:
            operands.append(bass2jax.partition_id_tensor())
        return tuple(bass2jax._bass_exec_p.bind(
            *operands,
            out_avals=tuple(out_avals),
            in_names=tuple(all_in_names),
            out_names=tuple(out_names),
            lowering_input_output_aliases=(),
            sim_require_finite=True,
            sim_require_nnan=True,
            nc=nc,
        ))

    devices = jax.devices()[:NCORES]
    mesh = Mesh(np.asarray(devices), ("core",))
    donate = tuple(range(n_params, n_params + n_outs))
    sharded = jax.jit(
        shard_map(_body, mesh=mesh,
                  in_specs=(PartitionSpec("core"),) * (n_params + n_outs),
                  out_specs=(PartitionSpec("core"),) * n_outs,
                  check_rep=False),
        donate_argnums=donate, keep_unused=True)

    def run(in_maps, device_inputs=None):
        if device_inputs is None:
            device_inputs = [
                np.concatenate([np.asarray(m[name]) for m in in_maps], axis=0)
                for name in in_names]
        zeros = [np.zeros((NCORES * a.shape[0], *a.shape[1:]), a.dtype)
                 for a in out_avals]
        out_arrs = sharded(*device_inputs, *zeros)
        return [
            {name: np.asarray(out_arrs[i]).reshape(NCORES, *out_avals[i].shape)[c]
             for i, name in enumerate(out_names)}
            for c in range(NCORES)
        ]

    run.in_names = in_names
    run.mesh = mesh
    return run


def _get_runner(reps=1, loop_iters=None):
    key = ("runner", reps, loop_iters)
    if key not in _CACHE:
        _CACHE[key] = _make_runner(_get_nc(reps, loop_iters))
    return _CACHE[key]


class _Res:
    def __init__(self, results):
        self.results = results


def run_on_hw(in_maps, reps=1, device_inputs=None, loop_iters=None):
    runner = _get_runner(reps, loop_iters)
    return _Res(runner(in_maps, device_inputs=device_inputs))


def kernel(node_embeddings, W1, b1, W2, b2, idx_prio, idx_rest,
           neg_idx_prio, neg_idx_rest, num_views=2, num_layers=3):
    in_maps = make_in_maps(node_embeddings, W1, b1, W2, b2, idx_prio, idx_rest,
                           neg_idx_prio, neg_idx_rest)
    res = run_on_hw(in_maps)
    _CACHE["last_results"] = res
    total = sum(float(res.results[c]["out"].sum()) for c in range(NCORES))
    return np.float32(total / COUNT)
